# revision 18
# baseline (speedup 1.0000x reference)
"""DDALoss Trainium2 kernel (8 NeuronCores, data-parallel over batch).

Math (algebraically identical to the reference):
  g[n,c]     = 2*feat[n]@centers[c] - ||centers[c]||^2          (logits shifted
               by the row-constant ||feat[n]||^2, which cancels in softmax)
  lse[n]     = log(sum_c exp(g[n,c]))
  glab[n]    = g[n, label[n]]
  nll_sum    = sum_n (lse[n] - glab[n])
  S1         = sum(feat^2)
  centerloss = (S1 - sum_n glab[n]) / (2N)
  ddaloss    = nll_sum / (2N^2)
  loss       = LAMB*centerloss + GAMMA*ddaloss

Per-core schedule (batch-sharded: 512 rows/core, all 10240 padded classes):
  - csq row: stream natural-layout bf16 centers (fused 3-D DMAs), DVE
    TENSOR_TENSOR_REDUCE squares with scale=-0.5 -> csqn[:, ct], then a
    gpsimd cast-DMA flattens each [128, 8] block to a [1, 1024] bf16 row
    holding -csq/2 in class order.
  - PE: psum[n128, c1024] accumulates 4 K=128 bf16 passes of feat.T@centers.T
    plus one K=1 "ones x (-csq/2)" pass, so PSUM = cross - csq/2.
  - ACT: exp(2*psum) = exp(2cross - csq) with free accum_out giving the
    row-sum directly (no DVE in the main loop).
  - label term: indirect-DMA gather of centers rows (fp32) + TTR dot products.
  - output: [1,3] partials (nll_sum, glab_sum, S1); final combine on host.
"""

import sys

sys.path.insert(0, "/opt/trn_rl_repo")

import numpy as np
import ml_dtypes

from contextlib import ExitStack

import concourse.bass as bass
import concourse.bacc as bacc
import concourse.tile as tile
from concourse import mybir

# Problem constants (hardcoded per harness contract)
N = 4096
D = 512
C = 10000
CP = 10240  # classes padded to 128*80
NCORES = 8
NPC = N // NCORES  # 512 rows per core
NT = NPC // 128  # 4 partition tiles per core
KT = D // 128  # 4 contraction blocks
CCH = 1024  # class chunk (psum tile free size)
NSCC = CP // CCH  # 10 chunks
NSUB = CCH // 512  # 2 matmuls of N=512 per chunk
JT = CCH // 128  # 8 class sub-tiles per chunk for csq

LAMB = 0.01
GAMMA = 3.0

BF16 = mybir.dt.bfloat16
FP8 = mybir.dt.float8e4
FP8E5 = mybir.dt.float8e5
F32 = mybir.dt.float32
I32 = mybir.dt.int32

# fp8 scaling: feat*FS and centers*CS on host keep e4m3 values in the normal
# range; psum then holds FS*CS*cross, the bias row holds -(FS*CS/2)*csq, and
# ACT's exp scale of 2/(FS*CS) restores exp(2*cross - csq).
FS = 8.0
CS = 16.0

_CACHE = {}


def _patch_ldw_opt():
    """bir_verify_and_optimise hardcodes --enable-ldw-opt=false, which makes
    walrus emit a weight reload before every matmul (+25% PE time here).
    Rewrite the flag; correctness is re-verified on hardware."""
    from concourse import bass_utils as _bu

    if getattr(_bu, "_ldw_patched", False):
        return
    _orig = _bu.run_command

    def _patched(argv, **kw):
        argv = [
            "--enable-ldw-opt=true" if a == "--enable-ldw-opt=false" else a
            for a in argv
        ]
        return _orig(argv, **kw)

    # Disabled: bacc/tile emit explicit InstLdweights, which walrus rejects
    # under --enable-ldw-opt=true ("not compatible with LDW optimization").
    # _bu.run_command = _patched
    _bu._ldw_patched = True


def _ttr(nc, out, in0, in1, accum_out, init, scale=1.0):
    """accum_out = init + sum_free(in0 * in1 * scale); out = elementwise scratch.

    Custom-DVE TENSOR_TENSOR_REDUCE (body Src0*Src1*C1, accum seed C0) -- the
    legacy InstTensorTensorReduce ISA opcode does not compile in this walrus.
    """
    from concourse.dve_ops import TENSOR_TENSOR_REDUCE

    nc.vector._custom_dve(
        TENSOR_TENSOR_REDUCE,
        out=out,
        in0=in0,
        in1=in1,
        s0=init,
        s1=scale,
        accum_out=accum_out,
    )


def _build():
    _patch_ldw_opt()
    nc = bacc.Bacc("TRN2", target_bir_lowering=False, debug=False)

    # Per-core external inputs
    ftT = nc.dram_tensor("ftt", [D, NPC], FP8, kind="ExternalInput")  # feat slice^T
    fnat = nc.dram_tensor("fnat", [NPC, D], F32, kind="ExternalInput")  # feat slice
    lab = nc.dram_tensor("lab", [NPC, 1], I32, kind="ExternalInput")
    cT = nc.dram_tensor("ct", [D, CP], FP8, kind="ExternalInput")  # centers.T pad 0
    cnat = nc.dram_tensor("cnat", [CP, D], BF16, kind="ExternalInput")  # centers pad 1
    cfull = nc.dram_tensor("cfull", [C, D], F32, kind="ExternalInput")  # for gather
    out = nc.dram_tensor("out", [1, 3], F32, kind="ExternalOutput")
    csq_dram = nc.dram_tensor("csq_scratch", [CP // 128, 128], BF16, kind="Internal")

    with tile.TileContext(nc) as tc, ExitStack() as ctx:
        const = ctx.enter_context(tc.tile_pool(name="const", bufs=1))
        small = ctx.enter_context(tc.tile_pool(name="small", bufs=2))
        cnp = ctx.enter_context(tc.tile_pool(name="cnp", bufs=3))
        ctp = ctx.enter_context(tc.tile_pool(name="ctp", bufs=4))
        expp = ctx.enter_context(tc.tile_pool(name="expp", bufs=2))
        scrp = ctx.enter_context(tc.tile_pool(name="scrp", bufs=2))
        ps_small = ctx.enter_context(tc.tile_pool(name="ps_small", bufs=1, space="PSUM"))

        # ---- constants / persistent tiles ----
        ones_f = const.tile([128, 1], F32)
        nc.vector.memset(ones_f, 1.0)
        ones_b = const.tile([1, 128], BF16)
        nc.vector.memset(ones_b, 1.0)
        ft = const.tile([128, KT, NPC], FP8, tag="ft")
        nc.sync.dma_start(out=ft, in_=ftT.ap().rearrange("(k p) n -> p k n", p=128))

        csqn = const.tile([128, CP // 128], F32, tag="csqn")  # -csq/2, [c_lo, ct]
        csqrow = const.tile([1, CP], BF16, tag="csqrow")  # -(FS*CS/2)*csq, class order
        accg = const.tile([128, NT * NSCC], F32, tag="accg")  # ACT accum grid
        cl4 = const.tile([128, NT], F32, tag="cl4")
        cq4 = const.tile([128, NT], F32, tag="cq4")
        fsq4 = const.tile([128, NT], F32, tag="fsq4")
        fin3 = const.tile([128, 3], F32, tag="fin3")

        # ---- main loop over class chunks ----
        cnat_r = cnat.ap().rearrange("(x p) d -> p x d", p=128)  # [128, 80, 512]
        cT_r = cT.ap().rearrange("(k p) c -> p k c", p=128)  # [128, 4, CP]

        def emit_csq_chain(scc):
            # -0.5*||c||^2 for classes [scc*CCH, (scc+1)*CCH) -> csqrow slice
            cn = cnp.tile([128, JT, D], BF16, tag="cn")
            nc.sync.dma_start(out=cn, in_=cnat_r[:, scc * JT : (scc + 1) * JT, :])
            for j in range(JT):
                scr = scrp.tile([128, D], BF16, tag="csq_scr")
                _ttr(
                    nc,
                    scr,
                    cn[:, j, :],
                    cn[:, j, :],
                    csqn[:, scc * JT + j : scc * JT + j + 1],
                    0.0,
                    scale=-(FS * CS / 2.0),
                )
            # flatten [128, JT] f32 -> [1, CCH] bf16 row in class order:
            # gpsimd cast-DMA writes the block transposed into DRAM (2-byte
            # scattered writes, but tiny and on the otherwise-idle SWDGE path)
            nc.gpsimd.dma_start(
                out=csq_dram.ap()[scc * JT : (scc + 1) * JT, :].rearrange(
                    "j p -> p j"
                ),
                in_=csqn[:, scc * JT : (scc + 1) * JT],
            )
            nc.sync.dma_start(
                out=csqrow[:1, scc * CCH : (scc + 1) * CCH],
                in_=bass.AP(tensor=csq_dram, offset=scc * CCH, ap=[[0, 1], [1, CCH]]),
            )

        with tc.tile_pool(name="ps_g", bufs=3, space="PSUM") as ps_g:
            # first chunk's centersT, split per-k so the PE can start on k=0
            # before the rest of the prerequisites land
            ct0 = ctp.tile([128, KT, CCH], FP8, tag="ct_t")
            for k in range(KT):
                nc.sync.dma_start(
                    out=ct0[:, k : k + 1, :], in_=cT_r[:, k : k + 1, 0:CCH]
                )
            emit_csq_chain(0)
            emit_csq_chain(1)
            for scc in range(NSCC):
                if scc == 0:
                    ct_t = ct0
                else:
                    ct_t = ctp.tile([128, KT, CCH], FP8, tag="ct_t")
                    nc.sync.dma_start(
                        out=ct_t, in_=cT_r[:, :, scc * CCH : (scc + 1) * CCH]
                    )
                if scc + 2 < NSCC:
                    emit_csq_chain(scc + 2)

                for nt in range(NT):
                    g = ps_g.tile([128, CCH], F32, tag="g")
                    for k in range(0, KT, 2):
                        for s in range(NSUB):
                            nc.tensor.matmul(
                                out=g[:, s * 512 : (s + 1) * 512],
                                lhsT=ft[:, k : k + 2, nt * 128 : (nt + 1) * 128],
                                rhs=ct_t[:, k : k + 2, s * 512 : (s + 1) * 512],
                                start=(k == 0),
                                stop=False,
                                perf_mode=mybir.MatmulPerfMode.DoubleRow,
                            )
                    for s in range(NSUB):
                        nc.tensor.matmul(
                            out=g[:, s * 512 : (s + 1) * 512],
                            lhsT=ones_b[:1, :],
                            rhs=csqrow[:1, scc * CCH + s * 512 : scc * CCH + (s + 1) * 512],
                            start=False,
                            stop=True,
                        )
                    scr_e = expp.tile([128, CCH], BF16, tag="scr_e")
                    col = nt * NSCC + scc
                    nc.scalar.activation(
                        scr_e,
                        g,
                        mybir.ActivationFunctionType.Exp,
                        scale=2.0 / (FS * CS),
                        accum_out=accg[:, col : col + 1],
                    )

        # ---- label path (independent; emitted late, runs in loop gaps) ----
        for nt in range(NT):
            labt = small.tile([128, 1], I32, tag="labt")
            nc.sync.dma_start(out=labt, in_=lab.ap()[nt * 128 : (nt + 1) * 128, :])
            crows = small.tile([128, D], F32, tag="crows")
            nc.gpsimd.indirect_dma_start(
                out=crows,
                out_offset=None,
                in_=cfull.ap(),
                in_offset=bass.IndirectOffsetOnAxis(ap=labt[:, :1], axis=0),
            )
            fnt = small.tile([128, D], F32, tag="fnt")
            nc.sync.dma_start(out=fnt, in_=fnat.ap()[nt * 128 : (nt + 1) * 128, :])
            scr1 = scrp.tile([128, D], F32, tag="lab_scr")
            _ttr(nc, scr1, fnt, crows, cl4[:, nt : nt + 1], 0.0)
            scr2 = scrp.tile([128, D], F32, tag="lab_scr")
            _ttr(nc, scr2, crows, crows, cq4[:, nt : nt + 1], 0.0)
            scr3 = scrp.tile([128, D], F32, tag="lab_scr")
            _ttr(nc, scr3, fnt, fnt, fsq4[:, nt : nt + 1], 0.0)

        # ---- finals ----
        sumexp4 = small.tile([128, NT], F32, tag="sumexp4")
        nc.vector.reduce_sum(
            sumexp4,
            accg[:, :].rearrange("p (nt s) -> p nt s", s=NSCC),
            axis=mybir.AxisListType.X,
        )
        lse4 = small.tile([128, NT], F32, tag="lse4")
        nc.scalar.activation(lse4, sumexp4, mybir.ActivationFunctionType.Ln)
        glab4 = small.tile([128, NT], F32, tag="glab4")
        nc.vector.tensor_scalar_mul(glab4, cl4, 2.0)
        nc.vector.tensor_sub(glab4, glab4, cq4)
        nld4 = small.tile([128, NT], F32, tag="nld4")
        nc.vector.tensor_sub(nld4, lse4, glab4)
        nc.vector.reduce_sum(fin3[:, 0:1], nld4, axis=mybir.AxisListType.X)
        nc.vector.reduce_sum(fin3[:, 1:2], glab4, axis=mybir.AxisListType.X)
        nc.vector.reduce_sum(fin3[:, 2:3], fsq4, axis=mybir.AxisListType.X)
        fin_ps = ps_small.tile([1, 3], F32, tag="fin_ps")
        nc.tensor.matmul(out=fin_ps, lhsT=ones_f, rhs=fin3, start=True, stop=True)
        out_sb = small.tile([1, 3], F32, tag="out_sb")
        nc.scalar.copy(out_sb, fin_ps)
        nc.sync.dma_start(out=out.ap(), in_=out_sb)

    nc.compile()
    return nc


def _get_nc():
    if "nc" not in _CACHE:
        _CACHE["nc"] = _build()
    return _CACHE["nc"]


def make_in_maps(feat, label, centers):
    feat = np.ascontiguousarray(np.asarray(feat, dtype=np.float32))
    centers = np.ascontiguousarray(np.asarray(centers, dtype=np.float32))
    label = np.ascontiguousarray(np.asarray(label).astype(np.int32).reshape(N, 1))

    bf = ml_dtypes.bfloat16
    f8 = ml_dtypes.float8_e4m3
    cT_pad = np.zeros((D, CP), dtype=f8)
    cT_pad[:, :C] = (centers.T * CS).astype(f8)
    cnat_pad = np.ones((CP, D), dtype=bf)  # pad rows -> csq=512 -> exp(-512)=0
    cnat_pad[:C, :] = centers.astype(bf)
    featT = np.ascontiguousarray(feat.T * FS).astype(f8)  # [D, N]

    in_maps = []
    for i in range(NCORES):
        sl = slice(i * NPC, (i + 1) * NPC)
        in_maps.append(
            {
                "ftt": np.ascontiguousarray(featT[:, sl]),
                "fnat": np.ascontiguousarray(feat[sl]),
                "lab": np.ascontiguousarray(label[sl]),
                "ct": cT_pad,
                "cnat": cnat_pad,
                "cfull": centers,
            }
        )
    return in_maps


def combine(parts):
    nll_sum, glab_sum, s1 = np.asarray(parts, dtype=np.float64).sum(axis=0)
    centerloss = (s1 - glab_sum) / (2.0 * N)
    ddaloss = nll_sum / (2.0 * N * N)
    loss = LAMB * centerloss + GAMMA * ddaloss
    return loss, centerloss, ddaloss


def kernel(feat, label, centers):
    from concourse.bass_utils import run_bass_kernel_spmd

    in_maps = make_in_maps(feat, label, centers)
    nc = _get_nc()
    res = run_bass_kernel_spmd(nc, in_maps, core_ids=list(range(NCORES)))
    parts = [r["out"].reshape(3) for r in res.results]
    loss, centerloss, ddaloss = combine(parts)
    return (
        np.float32(loss),
        np.float32(centerloss),
        np.float32(ddaloss),
    )


# revision 19
# speedup vs baseline: 1.1287x; 1.1287x over previous
"""DDALoss Trainium2 kernel (8 NeuronCores, data-parallel over batch).

Math (algebraically identical to the reference):
  g[n,c]     = 2*feat[n]@centers[c] - ||centers[c]||^2          (logits shifted
               by the row-constant ||feat[n]||^2, which cancels in softmax)
  lse[n]     = log(sum_c exp(g[n,c]))
  glab[n]    = g[n, label[n]]
  nll_sum    = sum_n (lse[n] - glab[n])
  S1         = sum(feat^2)
  centerloss = (S1 - sum_n glab[n]) / (2N)
  ddaloss    = nll_sum / (2N^2)
  loss       = LAMB*centerloss + GAMMA*ddaloss

Per-core schedule (batch-sharded: 512 rows/core, all 10240 padded classes):
  - csq row: stream natural-layout bf16 centers (fused 3-D DMAs), DVE
    TENSOR_TENSOR_REDUCE squares with scale=-0.5 -> csqn[:, ct], then a
    gpsimd cast-DMA flattens each [128, 8] block to a [1, 1024] bf16 row
    holding -csq/2 in class order.
  - PE: psum[n128, c1024] accumulates 4 K=128 bf16 passes of feat.T@centers.T
    plus one K=1 "ones x (-csq/2)" pass, so PSUM = cross - csq/2.
  - ACT: exp(2*psum) = exp(2cross - csq) with free accum_out giving the
    row-sum directly (no DVE in the main loop).
  - label term: indirect-DMA gather of centers rows (fp32) + TTR dot products.
  - output: [1,3] partials (nll_sum, glab_sum, S1); final combine on host.
"""

import sys

sys.path.insert(0, "/opt/trn_rl_repo")

import numpy as np
import ml_dtypes

from contextlib import ExitStack

import concourse.bass as bass
import concourse.bacc as bacc
import concourse.tile as tile
from concourse import mybir

# Problem constants (hardcoded per harness contract)
N = 4096
D = 512
C = 10000
CP = 10240  # classes padded to 128*80
NCORES = 8
NPC = N // NCORES  # 512 rows per core
NT = NPC // 128  # 4 partition tiles per core
KT = D // 128  # 4 contraction blocks
CCH = 1024  # class chunk (psum tile free size)
NSCC = CP // CCH  # 10 chunks
NSUB = CCH // 512  # 2 matmuls of N=512 per chunk
JT = CCH // 128  # 8 class sub-tiles per chunk for csq

LAMB = 0.01
GAMMA = 3.0

BF16 = mybir.dt.bfloat16
FP8 = mybir.dt.float8e4
FP8E5 = mybir.dt.float8e5
F32 = mybir.dt.float32
I32 = mybir.dt.int32

# fp8 scaling: feat*FS and centers*CS on host keep e4m3 values in the normal
# range; psum then holds FS*CS*cross, the bias row holds -(FS*CS/2)*csq, and
# ACT's exp scale of 2/(FS*CS) restores exp(2*cross - csq).
FS = 8.0
CS = 16.0

_CACHE = {}


def _patch_ldw_opt():
    """bir_verify_and_optimise hardcodes --enable-ldw-opt=false, which makes
    walrus emit a weight reload before every matmul (+25% PE time here).
    Rewrite the flag; correctness is re-verified on hardware."""
    from concourse import bass_utils as _bu

    if getattr(_bu, "_ldw_patched", False):
        return
    _orig = _bu.run_command

    def _patched(argv, **kw):
        argv = [
            "--enable-ldw-opt=true" if a == "--enable-ldw-opt=false" else a
            for a in argv
        ]
        return _orig(argv, **kw)

    # Disabled: bacc/tile emit explicit InstLdweights, which walrus rejects
    # under --enable-ldw-opt=true ("not compatible with LDW optimization").
    # _bu.run_command = _patched
    _bu._ldw_patched = True


def _ttr(nc, out, in0, in1, accum_out, init, scale=1.0):
    """accum_out = init + sum_free(in0 * in1 * scale); out = elementwise scratch.

    Custom-DVE TENSOR_TENSOR_REDUCE (body Src0*Src1*C1, accum seed C0) -- the
    legacy InstTensorTensorReduce ISA opcode does not compile in this walrus.
    """
    from concourse.dve_ops import TENSOR_TENSOR_REDUCE

    nc.vector._custom_dve(
        TENSOR_TENSOR_REDUCE,
        out=out,
        in0=in0,
        in1=in1,
        s0=init,
        s1=scale,
        accum_out=accum_out,
    )


def _build():
    _patch_ldw_opt()
    nc = bacc.Bacc("TRN2", target_bir_lowering=False, debug=False)

    # Per-core external inputs
    ftT = nc.dram_tensor("ftt", [D, NPC], FP8, kind="ExternalInput")  # feat slice^T
    fnat = nc.dram_tensor("fnat", [NPC, D], F32, kind="ExternalInput")  # feat slice
    lab = nc.dram_tensor("lab", [NPC, 1], I32, kind="ExternalInput")
    cT = nc.dram_tensor("ct", [D, CP], FP8, kind="ExternalInput")  # centers.T pad 0
    cnat = nc.dram_tensor("cnat", [CP, D], BF16, kind="ExternalInput")  # centers pad 1
    cfull = nc.dram_tensor("cfull", [C, D], F32, kind="ExternalInput")  # for gather
    out = nc.dram_tensor("out", [1, 3], F32, kind="ExternalOutput")
    csq_dram = nc.dram_tensor("csq_scratch", [CP // 128, 128], BF16, kind="Internal")

    with tile.TileContext(nc) as tc, ExitStack() as ctx:
        const = ctx.enter_context(tc.tile_pool(name="const", bufs=1))
        small = ctx.enter_context(tc.tile_pool(name="small", bufs=2))
        cnp = ctx.enter_context(tc.tile_pool(name="cnp", bufs=3))
        ctp = ctx.enter_context(tc.tile_pool(name="ctp", bufs=4))
        expp = ctx.enter_context(tc.tile_pool(name="expp", bufs=2))
        scrp = ctx.enter_context(tc.tile_pool(name="scrp", bufs=2))
        ps_small = ctx.enter_context(tc.tile_pool(name="ps_small", bufs=1, space="PSUM"))

        # ---- constants / persistent tiles ----
        ones_f = const.tile([128, 1], F32)
        nc.vector.memset(ones_f, 1.0)
        ones_b = const.tile([1, 128], BF16)
        nc.vector.memset(ones_b, 1.0)
        ident = const.tile([128, 128], F32, tag="ident")
        from concourse.masks import make_identity

        make_identity(nc, ident)

        ft = const.tile([128, KT, NPC], FP8, tag="ft")
        nc.sync.dma_start(out=ft, in_=ftT.ap().rearrange("(k p) n -> p k n", p=128))

        csqn = const.tile([128, CP // 128], F32, tag="csqn")  # -csq/2, [c_lo, ct]
        csqrow = const.tile([1, CP], BF16, tag="csqrow")  # -(FS*CS/2)*csq, class order
        accg = const.tile([128, NT * NSCC], F32, tag="accg")  # ACT accum grid
        cl4 = const.tile([128, NT], F32, tag="cl4")
        cq4 = const.tile([128, NT], F32, tag="cq4")
        fsq4 = const.tile([128, NT], F32, tag="fsq4")
        fin3 = const.tile([128, 3], F32, tag="fin3")

        # ---- main loop over class chunks ----
        cnat_r = cnat.ap().rearrange("(x p) d -> p x d", p=128)  # [128, 80, 512]
        cT_r = cT.ap().rearrange("(k p) c -> p k c", p=128)  # [128, 4, CP]

        def emit_csq_chain(scc):
            # -0.5*||c||^2 for classes [scc*CCH, (scc+1)*CCH) -> csqrow slice
            cn = cnp.tile([128, JT, D], BF16, tag="cn")
            nc.sync.dma_start(out=cn, in_=cnat_r[:, scc * JT : (scc + 1) * JT, :])
            for j in range(JT):
                scr = scrp.tile([128, D], BF16, tag="csq_scr")
                _ttr(
                    nc,
                    scr,
                    cn[:, j, :],
                    cn[:, j, :],
                    csqn[:, scc * JT + j : scc * JT + j + 1],
                    0.0,
                    scale=-(FS * CS / 2.0),
                )
            # flatten [128, JT] f32 -> [1, CCH] bf16 row in class order:
            # PE transpose to [JT, 128] psum, cast-DMA to DRAM, read back
            tp = ps_small.tile([JT, 128], F32, tag="tp")
            nc.tensor.transpose(
                out=tp, in_=csqn[:, scc * JT : (scc + 1) * JT], identity=ident
            )
            tp_sb = small.tile([JT, 128], BF16, tag="tp_sb")
            nc.vector.tensor_copy(tp_sb, tp)
            nc.sync.dma_start(
                out=csq_dram.ap()[scc * JT : (scc + 1) * JT, :], in_=tp_sb
            )
            nc.sync.dma_start(
                out=csqrow[:1, scc * CCH : (scc + 1) * CCH],
                in_=bass.AP(tensor=csq_dram, offset=scc * CCH, ap=[[0, 1], [1, CCH]]),
            )

        with tc.tile_pool(name="ps_g", bufs=3, space="PSUM") as ps_g:
            # first chunk's centersT, split per-k so the PE can start on k=0
            # before the rest of the prerequisites land
            ct0 = ctp.tile([128, KT, CCH], FP8, tag="ct_t")
            for k in range(KT):
                nc.sync.dma_start(
                    out=ct0[:, k : k + 1, :], in_=cT_r[:, k : k + 1, 0:CCH]
                )
            emit_csq_chain(0)
            emit_csq_chain(1)
            for scc in range(NSCC):
                if scc == 0:
                    ct_t = ct0
                else:
                    ct_t = ctp.tile([128, KT, CCH], FP8, tag="ct_t")
                    nc.sync.dma_start(
                        out=ct_t, in_=cT_r[:, :, scc * CCH : (scc + 1) * CCH]
                    )
                if scc + 2 < NSCC:
                    emit_csq_chain(scc + 2)

                for nt in range(NT):
                    g = ps_g.tile([128, CCH], F32, tag="g")
                    for k in range(0, KT, 2):
                        for s in range(NSUB):
                            nc.tensor.matmul(
                                out=g[:, s * 512 : (s + 1) * 512],
                                lhsT=ft[:, k : k + 2, nt * 128 : (nt + 1) * 128],
                                rhs=ct_t[:, k : k + 2, s * 512 : (s + 1) * 512],
                                start=(k == 0),
                                stop=False,
                                perf_mode=mybir.MatmulPerfMode.DoubleRow,
                            )
                    for s in range(NSUB):
                        nc.tensor.matmul(
                            out=g[:, s * 512 : (s + 1) * 512],
                            lhsT=ones_b[:1, :],
                            rhs=csqrow[:1, scc * CCH + s * 512 : scc * CCH + (s + 1) * 512],
                            start=False,
                            stop=True,
                        )
                    scr_e = expp.tile([128, CCH], BF16, tag="scr_e")
                    col = nt * NSCC + scc
                    nc.scalar.activation(
                        scr_e,
                        g,
                        mybir.ActivationFunctionType.Exp,
                        scale=2.0 / (FS * CS),
                        accum_out=accg[:, col : col + 1],
                    )

        # ---- label path (independent; emitted late, runs in loop gaps) ----
        for nt in range(NT):
            labt = small.tile([128, 1], I32, tag="labt")
            nc.sync.dma_start(out=labt, in_=lab.ap()[nt * 128 : (nt + 1) * 128, :])
            crows = small.tile([128, D], F32, tag="crows")
            nc.gpsimd.indirect_dma_start(
                out=crows,
                out_offset=None,
                in_=cfull.ap(),
                in_offset=bass.IndirectOffsetOnAxis(ap=labt[:, :1], axis=0),
            )
            fnt = small.tile([128, D], F32, tag="fnt")
            nc.sync.dma_start(out=fnt, in_=fnat.ap()[nt * 128 : (nt + 1) * 128, :])
            scr1 = scrp.tile([128, D], F32, tag="lab_scr")
            _ttr(nc, scr1, fnt, crows, cl4[:, nt : nt + 1], 0.0)
            scr2 = scrp.tile([128, D], F32, tag="lab_scr")
            _ttr(nc, scr2, crows, crows, cq4[:, nt : nt + 1], 0.0)
            scr3 = scrp.tile([128, D], F32, tag="lab_scr")
            _ttr(nc, scr3, fnt, fnt, fsq4[:, nt : nt + 1], 0.0)

        # ---- finals ----
        sumexp4 = small.tile([128, NT], F32, tag="sumexp4")
        nc.vector.reduce_sum(
            sumexp4,
            accg[:, :].rearrange("p (nt s) -> p nt s", s=NSCC),
            axis=mybir.AxisListType.X,
        )
        lse4 = small.tile([128, NT], F32, tag="lse4")
        nc.scalar.activation(lse4, sumexp4, mybir.ActivationFunctionType.Ln)
        glab4 = small.tile([128, NT], F32, tag="glab4")
        nc.vector.tensor_scalar_mul(glab4, cl4, 2.0)
        nc.vector.tensor_sub(glab4, glab4, cq4)
        nld4 = small.tile([128, NT], F32, tag="nld4")
        nc.vector.tensor_sub(nld4, lse4, glab4)
        nc.vector.reduce_sum(fin3[:, 0:1], nld4, axis=mybir.AxisListType.X)
        nc.vector.reduce_sum(fin3[:, 1:2], glab4, axis=mybir.AxisListType.X)
        nc.vector.reduce_sum(fin3[:, 2:3], fsq4, axis=mybir.AxisListType.X)
        fin_ps = ps_small.tile([1, 3], F32, tag="fin_ps")
        nc.tensor.matmul(out=fin_ps, lhsT=ones_f, rhs=fin3, start=True, stop=True)
        out_sb = small.tile([1, 3], F32, tag="out_sb")
        nc.scalar.copy(out_sb, fin_ps)
        nc.sync.dma_start(out=out.ap(), in_=out_sb)

    nc.compile()
    return nc


def _get_nc():
    if "nc" not in _CACHE:
        _CACHE["nc"] = _build()
    return _CACHE["nc"]


def make_in_maps(feat, label, centers):
    feat = np.ascontiguousarray(np.asarray(feat, dtype=np.float32))
    centers = np.ascontiguousarray(np.asarray(centers, dtype=np.float32))
    label = np.ascontiguousarray(np.asarray(label).astype(np.int32).reshape(N, 1))

    bf = ml_dtypes.bfloat16
    f8 = ml_dtypes.float8_e4m3
    cT_pad = np.zeros((D, CP), dtype=f8)
    cT_pad[:, :C] = (centers.T * CS).astype(f8)
    cnat_pad = np.ones((CP, D), dtype=bf)  # pad rows -> csq=512 -> exp(-512)=0
    cnat_pad[:C, :] = centers.astype(bf)
    featT = np.ascontiguousarray(feat.T * FS).astype(f8)  # [D, N]

    in_maps = []
    for i in range(NCORES):
        sl = slice(i * NPC, (i + 1) * NPC)
        in_maps.append(
            {
                "ftt": np.ascontiguousarray(featT[:, sl]),
                "fnat": np.ascontiguousarray(feat[sl]),
                "lab": np.ascontiguousarray(label[sl]),
                "ct": cT_pad,
                "cnat": cnat_pad,
                "cfull": centers,
            }
        )
    return in_maps


def combine(parts):
    nll_sum, glab_sum, s1 = np.asarray(parts, dtype=np.float64).sum(axis=0)
    centerloss = (s1 - glab_sum) / (2.0 * N)
    ddaloss = nll_sum / (2.0 * N * N)
    loss = LAMB * centerloss + GAMMA * ddaloss
    return loss, centerloss, ddaloss


def kernel(feat, label, centers):
    from concourse.bass_utils import run_bass_kernel_spmd

    in_maps = make_in_maps(feat, label, centers)
    nc = _get_nc()
    res = run_bass_kernel_spmd(nc, in_maps, core_ids=list(range(NCORES)))
    parts = [r["out"].reshape(3) for r in res.results]
    loss, centerloss, ddaloss = combine(parts)
    return (
        np.float32(loss),
        np.float32(centerloss),
        np.float32(ddaloss),
    )


# revision 20
# speedup vs baseline: 1.8270x; 1.6186x over previous
"""DDALoss Trainium2 kernel (8 NeuronCores, data-parallel over batch).

Math (algebraically identical to the reference):
  g[n,c]     = 2*feat[n]@centers[c] - ||centers[c]||^2          (logits shifted
               by the row-constant ||feat[n]||^2, which cancels in softmax)
  lse[n]     = log(sum_c exp(g[n,c]))
  glab[n]    = g[n, label[n]]
  nll_sum    = sum_n (lse[n] - glab[n])
  S1         = sum(feat^2)
  centerloss = (S1 - sum_n glab[n]) / (2N)
  ddaloss    = nll_sum / (2N^2)
  loss       = LAMB*centerloss + GAMMA*ddaloss

Per-core schedule (batch-sharded: 512 rows/core, all 10240 padded classes):
  - csq row: stream natural-layout bf16 centers (fused 3-D DMAs), DVE
    TENSOR_TENSOR_REDUCE squares with scale=-0.5 -> csqn[:, ct], then a
    gpsimd cast-DMA flattens each [128, 8] block to a [1, 1024] bf16 row
    holding -csq/2 in class order.
  - PE: psum[n128, c1024] accumulates 4 K=128 bf16 passes of feat.T@centers.T
    plus one K=1 "ones x (-csq/2)" pass, so PSUM = cross - csq/2.
  - ACT: exp(2*psum) = exp(2cross - csq) with free accum_out giving the
    row-sum directly (no DVE in the main loop).
  - label term: indirect-DMA gather of centers rows (fp32) + TTR dot products.
  - output: [1,3] partials (nll_sum, glab_sum, S1); final combine on host.
"""

import sys

sys.path.insert(0, "/opt/trn_rl_repo")

import numpy as np
import ml_dtypes

from contextlib import ExitStack

import concourse.bass as bass
import concourse.bacc as bacc
import concourse.tile as tile
from concourse import mybir

# Problem constants (hardcoded per harness contract)
N = 4096
D = 512
C = 10000
CP = 10240  # classes padded to 128*80
NCORES = 8
NPC = N // NCORES  # 512 rows per core
NT = NPC // 128  # 4 partition tiles per core
KT = D // 128  # 4 contraction blocks
CCH = 1024  # class chunk (psum tile free size)
NSCC = CP // CCH  # 10 chunks
NSUB = CCH // 512  # 2 matmuls of N=512 per chunk
JT = CCH // 128  # 8 class sub-tiles per chunk for csq

LAMB = 0.01
GAMMA = 3.0

BF16 = mybir.dt.bfloat16
FP8 = mybir.dt.float8e4
FP8E5 = mybir.dt.float8e5
F32 = mybir.dt.float32
I32 = mybir.dt.int32

# fp8 scaling: feat*FS and centers*CS on host keep e4m3 values in the normal
# range; psum then holds FS*CS*cross, the bias row holds -(FS*CS/2)*csq, and
# ACT's exp scale of 2/(FS*CS) restores exp(2*cross - csq).
FS = 8.0
CS = 16.0

_CACHE = {}


def _patch_ldw_opt():
    """bir_verify_and_optimise hardcodes --enable-ldw-opt=false, which makes
    walrus emit a weight reload before every matmul (+25% PE time here).
    Rewrite the flag; correctness is re-verified on hardware."""
    from concourse import bass_utils as _bu

    if getattr(_bu, "_ldw_patched", False):
        return
    _orig = _bu.run_command

    def _patched(argv, **kw):
        argv = [
            "--enable-ldw-opt=true" if a == "--enable-ldw-opt=false" else a
            for a in argv
        ]
        return _orig(argv, **kw)

    # Disabled: bacc/tile emit explicit InstLdweights, which walrus rejects
    # under --enable-ldw-opt=true ("not compatible with LDW optimization").
    # _bu.run_command = _patched
    _bu._ldw_patched = True


def _ttr(nc, out, in0, in1, accum_out, init, scale=1.0):
    """accum_out = init + sum_free(in0 * in1 * scale); out = elementwise scratch.

    Custom-DVE TENSOR_TENSOR_REDUCE (body Src0*Src1*C1, accum seed C0) -- the
    legacy InstTensorTensorReduce ISA opcode does not compile in this walrus.
    """
    from concourse.dve_ops import TENSOR_TENSOR_REDUCE

    nc.vector._custom_dve(
        TENSOR_TENSOR_REDUCE,
        out=out,
        in0=in0,
        in1=in1,
        s0=init,
        s1=scale,
        accum_out=accum_out,
    )


def _build():
    _patch_ldw_opt()
    nc = bacc.Bacc("TRN2", target_bir_lowering=False, debug=False)

    # Per-core external inputs
    ftT = nc.dram_tensor("ftt", [D, NPC], FP8, kind="ExternalInput")  # feat slice^T
    fnat = nc.dram_tensor("fnat", [NPC, D], F32, kind="ExternalInput")  # feat slice
    lab = nc.dram_tensor("lab", [NPC, 1], I32, kind="ExternalInput")
    cT = nc.dram_tensor("ct", [D, CP], FP8, kind="ExternalInput")  # centers.T pad 0
    cnat = nc.dram_tensor("cnat", [CP, D], BF16, kind="ExternalInput")  # centers pad 1
    cfull = nc.dram_tensor("cfull", [C, D], F32, kind="ExternalInput")  # for gather
    out = nc.dram_tensor("out", [1, 3], F32, kind="ExternalOutput")
    csq_dram = nc.dram_tensor("csq_scratch", [CP // 128, 128], BF16, kind="Internal")

    with tile.TileContext(nc) as tc, ExitStack() as ctx:
        const = ctx.enter_context(tc.tile_pool(name="const", bufs=1))
        small = ctx.enter_context(tc.tile_pool(name="small", bufs=2))
        cnp = ctx.enter_context(tc.tile_pool(name="cnp", bufs=3))
        ctp = ctx.enter_context(tc.tile_pool(name="ctp", bufs=4))
        expp = ctx.enter_context(tc.tile_pool(name="expp", bufs=2))
        scrp = ctx.enter_context(tc.tile_pool(name="scrp", bufs=2))
        ps_small = ctx.enter_context(tc.tile_pool(name="ps_small", bufs=1, space="PSUM"))

        # ---- constants / persistent tiles ----
        ones_f = const.tile([128, 1], F32)
        nc.vector.memset(ones_f, 1.0)
        ones_b = const.tile([1, 128], BF16)
        nc.vector.memset(ones_b, 1.0)
        ident = const.tile([128, 128], F32, tag="ident")
        from concourse.masks import make_identity

        make_identity(nc, ident)

        ft = const.tile([128, KT, NPC], FP8, tag="ft")
        nc.sync.dma_start(out=ft, in_=ftT.ap().rearrange("(k p) n -> p k n", p=128))

        csqn = const.tile([128, CP // 128], F32, tag="csqn")  # -csq/2, [c_lo, ct]
        csqrow = const.tile([1, CP], BF16, tag="csqrow")  # -(FS*CS/2)*csq, class order
        accg = const.tile([128, NT * NSCC], F32, tag="accg")  # ACT accum grid
        cl4 = const.tile([128, NT], F32, tag="cl4")
        cq4 = const.tile([128, NT], F32, tag="cq4")
        fsq4 = const.tile([128, NT], F32, tag="fsq4")
        fin3 = const.tile([128, 3], F32, tag="fin3")

        # ---- main loop over class chunks ----
        cnat_r = cnat.ap().rearrange("(x p) d -> p x d", p=128)  # [128, 80, 512]
        cT_r = cT.ap().rearrange("(k p) c -> p k c", p=128)  # [128, 4, CP]

        def emit_csq_chain(scc):
            # -0.5*||c||^2 for classes [scc*CCH, (scc+1)*CCH) -> csqrow slice
            cn = cnp.tile([128, JT, D], BF16, tag="cn")
            nc.sync.dma_start(out=cn, in_=cnat_r[:, scc * JT : (scc + 1) * JT, :])
            for j in range(JT):
                scr = scrp.tile([128, D], BF16, tag="csq_scr")
                _ttr(
                    nc,
                    scr,
                    cn[:, j, :],
                    cn[:, j, :],
                    csqn[:, scc * JT + j : scc * JT + j + 1],
                    0.0,
                    scale=-(FS * CS / 2.0),
                )
            # flatten [128, JT] f32 -> [1, CCH] bf16 row in class order:
            # PE transpose to [JT, 128] psum, cast-DMA to DRAM, read back
            tp = ps_small.tile([JT, 128], F32, tag="tp")
            nc.tensor.transpose(
                out=tp, in_=csqn[:, scc * JT : (scc + 1) * JT], identity=ident
            )
            tp_sb = small.tile([JT, 128], BF16, tag="tp_sb")
            nc.vector.tensor_copy(tp_sb, tp)
            nc.sync.dma_start(
                out=csq_dram.ap()[scc * JT : (scc + 1) * JT, :], in_=tp_sb
            )
            nc.sync.dma_start(
                out=csqrow[:1, scc * CCH : (scc + 1) * CCH],
                in_=bass.AP(tensor=csq_dram, offset=scc * CCH, ap=[[0, 1], [1, CCH]]),
            )

        with tc.tile_pool(name="ps_g", bufs=3, space="PSUM") as ps_g:
            emit_csq_chain(0)
            emit_csq_chain(1)
            for scc in range(NSCC):
                ct_t = ctp.tile([128, KT, CCH], FP8, tag="ct_t")
                nc.sync.dma_start(
                    out=ct_t, in_=cT_r[:, :, scc * CCH : (scc + 1) * CCH]
                )
                if scc + 2 < NSCC:
                    emit_csq_chain(scc + 2)

                for nt in range(NT):
                    g = ps_g.tile([128, CCH], F32, tag="g")
                    for k in range(0, KT, 2):
                        for s in range(NSUB):
                            nc.tensor.matmul(
                                out=g[:, s * 512 : (s + 1) * 512],
                                lhsT=ft[:, k : k + 2, nt * 128 : (nt + 1) * 128],
                                rhs=ct_t[:, k : k + 2, s * 512 : (s + 1) * 512],
                                start=(k == 0),
                                stop=False,
                                perf_mode=mybir.MatmulPerfMode.DoubleRow,
                            )
                    for s in range(NSUB):
                        nc.tensor.matmul(
                            out=g[:, s * 512 : (s + 1) * 512],
                            lhsT=ones_b[:1, :],
                            rhs=csqrow[:1, scc * CCH + s * 512 : scc * CCH + (s + 1) * 512],
                            start=False,
                            stop=True,
                        )
                    scr_e = expp.tile([128, CCH], BF16, tag="scr_e")
                    col = nt * NSCC + scc
                    nc.scalar.activation(
                        scr_e,
                        g,
                        mybir.ActivationFunctionType.Exp,
                        scale=2.0 / (FS * CS),
                        accum_out=accg[:, col : col + 1],
                    )

        # ---- label path (independent; emitted late, runs in loop gaps) ----
        for nt in range(NT):
            labt = small.tile([128, 1], I32, tag="labt")
            nc.sync.dma_start(out=labt, in_=lab.ap()[nt * 128 : (nt + 1) * 128, :])
            crows = small.tile([128, D], F32, tag="crows")
            nc.gpsimd.indirect_dma_start(
                out=crows,
                out_offset=None,
                in_=cfull.ap(),
                in_offset=bass.IndirectOffsetOnAxis(ap=labt[:, :1], axis=0),
            )
            fnt = small.tile([128, D], F32, tag="fnt")
            nc.sync.dma_start(out=fnt, in_=fnat.ap()[nt * 128 : (nt + 1) * 128, :])
            scr1 = scrp.tile([128, D], F32, tag="lab_scr")
            _ttr(nc, scr1, fnt, crows, cl4[:, nt : nt + 1], 0.0)
            scr2 = scrp.tile([128, D], F32, tag="lab_scr")
            _ttr(nc, scr2, crows, crows, cq4[:, nt : nt + 1], 0.0)
            scr3 = scrp.tile([128, D], F32, tag="lab_scr")
            _ttr(nc, scr3, fnt, fnt, fsq4[:, nt : nt + 1], 0.0)

        # ---- finals ----
        sumexp4 = small.tile([128, NT], F32, tag="sumexp4")
        nc.vector.reduce_sum(
            sumexp4,
            accg[:, :].rearrange("p (nt s) -> p nt s", s=NSCC),
            axis=mybir.AxisListType.X,
        )
        lse4 = small.tile([128, NT], F32, tag="lse4")
        nc.scalar.activation(lse4, sumexp4, mybir.ActivationFunctionType.Ln)
        glab4 = small.tile([128, NT], F32, tag="glab4")
        nc.vector.tensor_scalar_mul(glab4, cl4, 2.0)
        nc.vector.tensor_sub(glab4, glab4, cq4)
        nld4 = small.tile([128, NT], F32, tag="nld4")
        nc.vector.tensor_sub(nld4, lse4, glab4)
        nc.vector.reduce_sum(fin3[:, 0:1], nld4, axis=mybir.AxisListType.X)
        nc.vector.reduce_sum(fin3[:, 1:2], glab4, axis=mybir.AxisListType.X)
        nc.vector.reduce_sum(fin3[:, 2:3], fsq4, axis=mybir.AxisListType.X)
        fin_ps = ps_small.tile([1, 3], F32, tag="fin_ps")
        nc.tensor.matmul(out=fin_ps, lhsT=ones_f, rhs=fin3, start=True, stop=True)
        out_sb = small.tile([1, 3], F32, tag="out_sb")
        nc.scalar.copy(out_sb, fin_ps)
        nc.sync.dma_start(out=out.ap(), in_=out_sb)

    nc.compile()
    return nc


def _get_nc():
    if "nc" not in _CACHE:
        _CACHE["nc"] = _build()
    return _CACHE["nc"]


def make_in_maps(feat, label, centers):
    feat = np.ascontiguousarray(np.asarray(feat, dtype=np.float32))
    centers = np.ascontiguousarray(np.asarray(centers, dtype=np.float32))
    label = np.ascontiguousarray(np.asarray(label).astype(np.int32).reshape(N, 1))

    bf = ml_dtypes.bfloat16
    f8 = ml_dtypes.float8_e4m3
    cT_pad = np.zeros((D, CP), dtype=f8)
    cT_pad[:, :C] = (centers.T * CS).astype(f8)
    cnat_pad = np.ones((CP, D), dtype=bf)  # pad rows -> csq=512 -> exp(-512)=0
    cnat_pad[:C, :] = centers.astype(bf)
    featT = np.ascontiguousarray(feat.T * FS).astype(f8)  # [D, N]

    in_maps = []
    for i in range(NCORES):
        sl = slice(i * NPC, (i + 1) * NPC)
        in_maps.append(
            {
                "ftt": np.ascontiguousarray(featT[:, sl]),
                "fnat": np.ascontiguousarray(feat[sl]),
                "lab": np.ascontiguousarray(label[sl]),
                "ct": cT_pad,
                "cnat": cnat_pad,
                "cfull": centers,
            }
        )
    return in_maps


def combine(parts):
    nll_sum, glab_sum, s1 = np.asarray(parts, dtype=np.float64).sum(axis=0)
    centerloss = (s1 - glab_sum) / (2.0 * N)
    ddaloss = nll_sum / (2.0 * N * N)
    loss = LAMB * centerloss + GAMMA * ddaloss
    return loss, centerloss, ddaloss


def kernel(feat, label, centers):
    from concourse.bass_utils import run_bass_kernel_spmd

    in_maps = make_in_maps(feat, label, centers)
    nc = _get_nc()
    res = run_bass_kernel_spmd(nc, in_maps, core_ids=list(range(NCORES)))
    parts = [r["out"].reshape(3) for r in res.results]
    loss, centerloss, ddaloss = combine(parts)
    return (
        np.float32(loss),
        np.float32(centerloss),
        np.float32(ddaloss),
    )
